# revision 38
# baseline (speedup 1.0000x reference)
"""Trainium2 Bass kernel for nn_BayerFeatureExtractor.

Computes 52 feature channels from a [2,1,768,768] bayer image, data-parallel
over 8 NeuronCores (each core: one batch image x 192 rows, 2 row-blocks).

Strategy (v2, DMA/engine-balanced):
  - Host reflect-pads each batch image by (3 rows, 6 cols); each core gets a
    [198, 780] fp32r strip (rows on SBUF partitions).
  - ALL linear filters (2D stencils, 1xN/Nx1 kernels, 3x3 boxes, polyphase
    masked smooth5) run on the TensorEngine as banded matmuls: contraction
    over input rows with a banded weight matrix encoding the kernel's column
    profile; one PSUM-accumulated pass per nonzero kernel column with the
    moving operand shifted along the free dim.
  - 3x3 boxes of intermediates run on an "E" grid extended by 1 row/col so
    the second conv stage needs no partition-offset reads; reflect behavior
    at borders is exact (profiles symmetric or squared); gxy's antisymmetric
    reflect fixed by sign flips + per-edge-core band variants.
  - Pointwise math spread across Vector (DVE), Scalar (ACT), GPSIMD (Pool).
  - Morphological gradient via 3 row-shifted DMA copies + max/min chains.
  - Adjacent-channel groups that finish together are written into multi-
    channel SBUF pack tiles and go out in one strided-AP DMA each (row-major
    over channels); stragglers DMA individually straight from their compute
    buffers. The 7 constant mask channels go out in one DRAM->DRAM DMA from
    a host [7,192,768] tensor.
"""
import sys
import math

sys.path.insert(0, '/opt/trn_rl_repo')

import numpy as np

EPS = 1e-6

H = 768
W = 768
B = 2
NCORES = 8
CORES_PER_BATCH = 4
RPC = H // CORES_PER_BATCH          # 192 output rows per core
NBLK = 2
BR = RPC // NBLK                    # 96 output rows per block
PH = 3                              # host row padding
PW = 6                              # host col padding
SR = RPC + 2 * PH                   # 198 strip rows
SW = W + 2 * PW                     # 780 strip cols
KIN = BR + 2 * PH                   # 102 contraction rows per block
ME, MO = 98, 96                     # E-grid / O-grid matmul M
NE, NO = 386, 384                   # matmul half widths
EW = 772                            # E tile width (covers out cols -2..769)


# ---------------------------------------------------------------- kernels ---
def _npk(a, s=1.0):
    return np.asarray(a, dtype=np.float32) * np.float32(s)


def _gabor(theta, sigma=1.1, lambd=3.0, gamma=0.65):
    c = np.arange(-2, 3, dtype=np.float32)
    yy, xx = np.meshgrid(c, c, indexing='ij')
    xt = xx * math.cos(theta) + yy * math.sin(theta)
    yt = -xx * math.sin(theta) + yy * math.cos(theta)
    k = np.exp(-(xt ** 2 + gamma ** 2 * yt ** 2) / (2.0 * sigma ** 2)) * np.cos(
        2.0 * math.pi * xt / lambd)
    k = k - k.mean()
    return (k / max(np.abs(k).sum(), 1e-6)).astype(np.float32)


def _dct_like(u=2, v=2, size=5):
    c = np.arange(size, dtype=np.float32)
    yy, xx = np.meshgrid(c, c, indexing='ij')
    k = np.cos(math.pi * (2 * xx + 1) * u / (2 * size)) * np.cos(
        math.pi * (2 * yy + 1) * v / (2 * size))
    k = k - k.mean()
    return (k / max(np.abs(k).sum(), 1e-6)).astype(np.float32)


K_LAP = _npk([[0, 1, 0], [1, -4, 1], [0, 1, 0]])
K_HXX = _npk([[1, -2, 1]])                     # 1x3 row kernel
K_HYY = _npk([[1], [-2], [1]])                 # 3x1 col kernel
K_HXY = _npk([[1, 0, -1], [0, 0, 0], [-1, 0, 1]], 0.25)
K_GX = _npk([[-1, 0, 1], [-2, 0, 2], [-1, 0, 1]], 0.125)
K_GY = _npk([[-1, -2, -1], [0, 0, 0], [1, 2, 1]], 0.125)
K_GDM = _npk([[-2, -1, 0], [-1, 0, 1], [0, 1, 2]], 0.125)
K_GDA = _npk([[0, 1, 2], [-1, 0, 1], [-2, -1, 0]], 0.125)
K_CHK = _npk([[1, -1, 1], [-1, 1, -1], [1, -1, 1]], 1.0 / 9.0)

HGH_V = np.array([-0.25, 0.5, 0.5, 0.5, -0.25], np.float32)
K_HSH = _npk([[-0.5, 0.0, 1.0, 0.0, -0.5]])
K_HSV = _npk([[-0.5], [0.0], [1.0], [0.0], [-0.5]])
K_HGH = HGH_V.reshape(1, 5)
K_HGV = HGH_V.reshape(5, 1)
K_MHC = _npk([[0, 0, -1, 0, 0], [0, 0, 2, 0, 0], [-1, 2, 4, 2, -1],
              [0, 0, 2, 0, 0], [0, 0, -1, 0, 0]], 0.125)
K_STX = _npk([[0.25, -1.0, 1.5, -1.0, 0.25]])
K_STY = K_STX.reshape(5, 1).copy()
K_G45 = _gabor(math.pi / 4.0)
K_G135 = _gabor(3.0 * math.pi / 4.0)
K_DCT = _dct_like()
K_SMOOTH5 = (_npk([[1, 2, 3, 2, 1], [2, 4, 6, 4, 2], [3, 6, 9, 6, 3],
                   [2, 4, 6, 4, 2], [1, 2, 3, 2, 1]]) / np.float32(81.0))
K_RESH = (np.eye(1, 5, 2, dtype=np.float32) - K_HGH)      # delta - hgh (1x5)
K_RESV = (np.eye(5, 1, -2, dtype=np.float32) - K_HGV)     # delta - hgv (5x1)
K_BOX3 = np.full((3, 3), 1.0 / 9.0, np.float32)
K_AVGH5 = np.full((1, 5), 0.2, np.float32)
K_AVGV5 = np.full((5, 1), 0.2, np.float32)


def _mask_pattern(name):
    # value at (row parity, col parity), gbrg pattern
    m = np.zeros((2, 2), np.float32)
    if name == 'r':
        m[1, 0] = 1.0
    elif name == 'b':
        m[0, 1] = 1.0
    elif name == 'gr':
        m[1, 1] = 1.0
    elif name == 'gb':
        m[0, 0] = 1.0
    elif name == 'g':
        m[0, 0] = 1.0; m[1, 1] = 1.0
    elif name == 'row':
        m[1, :] = 1.0
    elif name == 'col':
        m[:, 1] = 1.0
    return m


def _den_pattern(name):
    # conv(mask, SMOOTH5) is exactly 2x2-periodic (reflect == parity ext.)
    pat = _mask_pattern(name)
    g = np.zeros((16, 16), np.float32)
    for r in range(16):
        for c in range(16):
            g[r, c] = pat[r % 2, c % 2]
    out = np.zeros((2, 2), np.float32)
    for r in (6, 7):
        for c in (6, 7):
            acc = np.float32(0.0)
            for dy in range(5):
                for dx in range(5):
                    acc += K_SMOOTH5[dy, dx] * g[r + dy - 2, c + dx - 2]
            out[r % 2, c % 2] = acc
    return np.maximum(out, EPS)


def _tile_pattern(pat, rows, cols):
    out = np.zeros((rows, cols), np.float32)
    for rp in range(2):
        for cp in range(2):
            out[rp::2, cp::2] = pat[rp % 2, cp % 2]
    return out


# ------------------------------------------------------------ band builder ---
class Bands:
    """Dedup banded lhsT matrices per grid ('O' out rows, 'E' extended, 'B' box)."""

    def __init__(self):
        self.items = {'O': [], 'E': [], 'B': []}
        self.index = {}

    def get(self, grid, prof):
        key = (grid, tuple(np.round(np.asarray(prof, np.float64), 10)))
        if key in self.index:
            return self.index[key]
        prof = np.asarray(prof, np.float32)
        kh = len(prof)
        off = kh // 2
        if grid == 'O':
            m = np.zeros((KIN, MO), np.float32)
            for mm in range(MO):
                base = mm + PH - off
                for t in range(kh):
                    m[base + t, mm] = prof[t]
        elif grid == 'E':
            m = np.zeros((KIN, ME), np.float32)
            for ii in range(ME):
                base = ii + PH - 1 - off
                for t in range(kh):
                    m[base + t, ii] = prof[t]
        else:  # 'B': 3-row box applied to E tiles
            m = np.zeros((ME, MO), np.float32)
            for mm in range(MO):
                for t in range(kh):
                    m[mm + t, mm] = prof[t]
        idx = len(self.items[grid])
        self.items[grid].append(m)
        self.index[key] = idx
        return idx

    def passes(self, grid, K):
        K = np.atleast_2d(np.asarray(K, np.float32))
        kw = K.shape[1]
        out = []
        for dxi in range(kw):
            col = K[:, dxi]
            if np.any(col != 0.0):
                out.append((dxi - kw // 2, self.get(grid, col)))
        return out


_BANDS = Bands()

P_E = {
    'gx': _BANDS.passes('E', K_GX),
    'gy': _BANDS.passes('E', K_GY),
    'chk': _BANDS.passes('E', K_CHK),
    'sty': _BANDS.passes('E', K_STY),
    'resv': _BANDS.passes('E', K_RESV),
    'stx': _BANDS.passes('E', K_STX),
    'resh': _BANDS.passes('E', K_RESH),
}
P_O = {
    'hyy': _BANDS.passes('O', K_HYY),
    'hxy': _BANDS.passes('O', K_HXY),
    'gdm': _BANDS.passes('O', K_GDM),
    'gda': _BANDS.passes('O', K_GDA),
    'hsv': _BANDS.passes('O', K_HSV),
    'hgv': _BANDS.passes('O', K_HGV),
    'g45': _BANDS.passes('O', K_G45),
    'g135': _BANDS.passes('O', K_G135),
    'dct': _BANDS.passes('O', K_DCT),
    'mhc': _BANDS.passes('O', K_MHC),
    'avgv5': _BANDS.passes('O', K_AVGV5),
    'box3': _BANDS.passes('O', K_BOX3),
    'lap': _BANDS.passes('O', K_LAP),
    'hxx': _BANDS.passes('O', K_HXX),
    'hsh': _BANDS.passes('O', K_HSH),
    'hgh': _BANDS.passes('O', K_HGH),
    'avgh5': _BANDS.passes('O', K_AVGH5),
}
def _trunc10(a):
    b = np.asarray(a, np.float32).copy()
    v = b.view(np.uint32)
    v &= np.uint32(0xFFFFE000)
    return b


K_AVGV5_H = _trunc10(K_AVGV5)
K_AVGV5_L = (K_AVGV5 - K_AVGV5_H).astype(np.float32)
P_O['avgv5_h'] = _BANDS.passes('O', K_AVGV5_H)
P_O['avgv5_l'] = _BANDS.passes('O', K_AVGV5_L)
_BANDS.get('B', K_BOX3[:, 0])
P_B3 = [(-1, 0), (0, 0), (1, 0)]   # 3-pass box over E tiles (band 0 = std)

BANDS_E = np.stack(_BANDS.items['E'])          # [nE, 102, 98]

# --- polyphase smooth5 bands: row-parity mask and 1/den folded into bands.
_SM5_META = {'r': (1, 0), 'b': (0, 1), 'gr': (1, 1), 'gb': (0, 0)}


def _sm5_passes(X, D):
    rX, cX = _SM5_META[X]
    rden = 1.0 / _den_pattern(D)
    out = []
    for p in range(2):
        for dxi in range(5):
            if (p + dxi) % 2 != cX:
                continue
            m = np.zeros((KIN, MO), np.float32)
            for mm in range(MO):
                for t in range(5):
                    k = mm + 1 + t
                    if (k + 1) % 2 == rX:
                        m[k, mm] = K_SMOOTH5[t, dxi] * rden[mm % 2, p]
            key = ('SM5', X, D, p, dxi)
            if key not in _BANDS.index:
                _BANDS.index[key] = len(_BANDS.items['O'])
                _BANDS.items['O'].append(m)
            # rhs strided-view offset (in cX-parity column units)
            j0 = (PW + p + (dxi - 2) - cX) // 2
            out.append((p, _BANDS.index[key], j0))
    return out


P_SM5 = {
    'rf': (_sm5_passes('r', 'r'), 0),
    'bf': (_sm5_passes('b', 'b'), 1),
    'grf': (_sm5_passes('gr', 'gr'), 1),
    'gbf': (_sm5_passes('gb', 'gb'), 0),
}
P_SM5_GF = (_sm5_passes('gr', 'g'), _sm5_passes('gb', 'g'))  # accumulate both

BANDS_O = np.stack(_BANDS.items['O'])          # [nO, 102, 96]
_BB_STD = _BANDS.items['B'][0]
BANDS_B = np.stack([_BB_STD, _BB_STD, _BB_STD])   # std, top-slot, bot-slot


def _bb_variant(kind):
    m = _BB_STD.copy()
    if kind == 'top':
        m[0, 0] = -m[0, 0]
    else:
        m[ME - 1, MO - 1] = -m[ME - 1, MO - 1]
    return m


CH = {n: i for i, n in enumerate([
    'r', 'g', 'b', 'gr', 'gb', 'rowm', 'colm',
    'lap', 'hxx', 'hyy', 'hxy', 'mgrad', 'gx', 'gy', 'gdm', 'gda', 'gmag',
    'coherence', 'anisotropy', 'hsh', 'hsv', 'hgh', 'hgv', 'ha_dis',
    'res_h', 'res_v', 'res_eh', 'res_ev', 'dgd', 'dsd', 'lvh', 'lvv', 'lvd',
    'dconf', 'rg', 'bg', 'gpd', 'mhc', 'mhc_ha', 'rres', 'bres',
    'stx', 'sty', 'chk', 'g45', 'g135', 'dctp', 'chk_e', 'str_e', 'lmean',
    'lvar', 'gen'])}

# O-grid packs: contiguous channel runs whose slots complete close together.
O_PACKS = {
    'p0711': ['lap', 'hxx', 'hyy', 'hxy'],               # ch 7..10
    'p2123': ['hgh', 'hgv', 'ha_dis'],                   # ch 21..23
    'p3033': ['lvh', 'lvv', 'lvd', 'dconf'],             # ch 30..33
    'p4446': ['g45', 'g135', 'dctp'],                    # ch 44..46
}
E_PACKS = {
    'egxy': ['gx', 'gy'],            # ch 12,13
    'estc': ['stx', 'sty', 'chk'],   # ch 41,42,43
}


# ------------------------------------------------------------- bass program ---
_PROGRAM = {}


def _build_program(loop=1, timing=False):
    import concourse.bacc as bacc
    import concourse.mybir as mybir
    from concourse.tile import TileContext

    f32 = mybir.dt.float32
    f32r = mybir.dt.float32r
    A = mybir.AluOpType
    AF = mybir.ActivationFunctionType

    nc = bacc.Bacc("TRN2")

    if timing:
        def declare(name, shape, dtype, isOutput):
            return nc.dram_tensor(name, shape, dtype).ap()
        tin = nc.declare_dram_parameter("tin", [1, 4], mybir.dt.float32, isOutput=False)
        tout = nc.declare_dram_parameter("tout", [1, 4], mybir.dt.float32, isOutput=True)
    else:
        def declare(name, shape, dtype, isOutput):
            return nc.declare_dram_parameter(name, shape, dtype, isOutput=isOutput)

    def register_const(value):
        t = nc.alloc_sbuf_tensor(f"constf32-{value}", [128, 1], f32)
        nc.gpsimd.memset(t.ap(), value)
        nc.const_aps.aps[(f32, value)] = t.ap()

    register_const(EPS)
    nc.all_engine_barrier()

    nO, nE, nB = BANDS_O.shape[0], BANDS_E.shape[0], BANDS_B.shape[0]
    xs_ext = declare("xs", [SR, SW], f32r, isOutput=False)
    bo_ext = declare("bandsO", [KIN, nO * MO], f32r, isOutput=False)
    be_ext = declare("bandsE", [KIN, nE * ME], f32r, isOutput=False)
    bb_ext = declare("bandsB", [ME, nB * MO], f32r, isOutput=False)
    rbgmask_ext = declare("rbgmask", [MO, 2 * W], f32, isOutput=False)
    masks14_ext = declare("masks14", [14, W], f32, isOutput=False)
    out_ext = declare("out", [52, RPC, W], f32, isOutput=True)

    with TileContext(nc) as tc:
        with (
            tc.tile_pool(name="const", bufs=1) as cpool,
            tc.tile_pool(name="work", bufs=1) as wpool,
            tc.tile_pool(name="ebuf", bufs=5) as epool,
            tc.tile_pool(name="obuf", bufs=6) as opool,
            tc.tile_pool(name="pack", bufs=1) as kpool,
            tc.tile_pool(name="psum", bufs=4, space="PSUM") as pps,
        ):
            # ---- constants -> SBUF (once, scalar ring; E bands first since
            # the E convs are the first consumers) ----
            be_t = cpool.tile([KIN, nE * ME], f32r)
            nc.scalar.dma_start(out=be_t[:], in_=be_ext[:])
            bb_t = cpool.tile([ME, nB * MO], f32r)
            nc.scalar.dma_start(out=bb_t[:], in_=bb_ext[:])
            bo_t = cpool.tile([KIN, nO * MO], f32r)
            nc.scalar.dma_start(out=bo_t[:], in_=bo_ext[:])
            rbg_t = cpool.tile([MO, 2 * W], f32)
            nc.scalar.dma_start(out=rbg_t[:], in_=rbgmask_ext[:])
            m14_t = cpool.tile([14, W], f32)
            nc.scalar.dma_start(out=m14_t[:], in_=masks14_ext[:])

            def bandO(i):
                return bo_t[:, i * MO:(i + 1) * MO]

            def bandE(i):
                return be_t[:, i * ME:(i + 1) * ME]

            def bandB(i):
                return bb_t[:, i * MO:(i + 1) * MO]

            def h3(ap):
                return ap.rearrange("p (b n) -> p b n", b=2)

            if timing:
                nc.sync.dma_start(out=tout[:], in_=tin[:])

            # DRAM view iterating (row, channel, col) over `k` channels
            # starting at ch0 with unit channel stride.
            def out_rcw(ch0, k, r0, nr):
                c = out_ext[ch0:ch0 + 1, r0:r0 + nr, 0:W].copy()
                c.ap = mybir.VecI64Pair([[W, nr], [RPC * W, k], [1, W]])
                return c

            def ring():
                # all output DMAs on the SP ring: SP has no compute, so a
                # DMA blocked waiting for its producer doesn't stall an
                # engine's instruction stream (scalar/vector rings would).
                return nc.sync

            import contextlib
            loop_cm = tc.For_i(0, loop, 1) if loop > 1 else contextlib.nullcontext()
            with loop_cm:
              # ---- inputs for BOTH blocks up front (double-buffered tags)
              # so block-1 loads aren't queued behind block-0 output DMAs
              # on the SP ring.
              intiles = []
              for blk in range(NBLK):
                  r0 = blk * BR
                  strip = wpool.tile([KIN, SW], f32r, tag="strip", bufs=2)
                  nc.sync.dma_start(out=strip[:], in_=xs_ext[r0:r0 + KIN, :])
                  T0e = wpool.tile([ME, SW], f32, tag="T0e", bufs=2)
                  nc.sync.dma_start(out=T0e[:], in_=strip[2:2 + ME, :].bitcast(f32))
                  T1 = wpool.tile([MO, SW], f32, tag="T1", bufs=2)
                  nc.sync.dma_start(out=T1[:], in_=strip[3:3 + MO, :].bitcast(f32))
                  T2 = wpool.tile([MO, SW], f32, tag="T2", bufs=2)
                  nc.sync.dma_start(out=T2[:], in_=strip[4:4 + MO, :].bitcast(f32))
                  intiles.append((strip, T0e, T1, T2))

              for blk in range(NBLK):
                  r0 = blk * BR
                  strip, T0e, T1, T2 = intiles[blk]
                  stripf = strip[:].bitcast(f32)
                  bayerO = T1[:, PW:PW + W]

                  bsqr = wpool.tile([KIN, SW], f32r, tag="bsqr", bufs=2)
                  nc.scalar.activation(bsqr[:], stripf, AF.Square)

                  # ---- output packs & slot views ----
                  oslot = {}
                  opack_t = {}
                  for name, chans in O_PACKS.items():
                      t = kpool.tile([MO, len(chans) * W], f32, tag=name,
                                     name=name)
                      v = t[:].rearrange("p (c w) -> p c w", c=len(chans))
                      v3 = t[:].rearrange("p (c b n) -> p c b n",
                                          c=len(chans), b=2)
                      opack_t[name] = (t, chans)
                      for s, ch in enumerate(chans):
                          oslot[ch] = (v[:, s, :], v3[:, s, :, :])
                  eslot = {}
                  epack_t = {}
                  for name, chans in E_PACKS.items():
                      t = kpool.tile([ME, len(chans) * EW], f32, tag=name,
                                     name=name)
                      v = t[:].rearrange("p (c w) -> p c w", c=len(chans))
                      v3 = t[:].rearrange("p (c b n) -> p c b n",
                                          c=len(chans), b=2)
                      epack_t[name] = (t, chans)
                      for s, ch in enumerate(chans):
                          eslot[ch] = (v[:, s, :], v3[:, s, :, :])

                  def o2(ch):
                      return oslot[ch][0]

                  def o3(ch):
                      return oslot[ch][1]

                  def e2(ch):
                      return eslot[ch][0]

                  def e3(ch):
                      return eslot[ch][1]

                  def dma_pack_o(name):
                      t, chans = opack_t[name]
                      v = t[:].rearrange("p (c w) -> p c w", c=len(chans))
                      ring().dma_start(out=out_rcw(CH[chans[0]], len(chans), r0, BR),
                                       in_=v)

                  def dma_pack_e(name):
                      t, chans = epack_t[name]
                      v = t[:].rearrange("p (c w) -> p c w", c=len(chans))
                      ring().dma_start(out=out_rcw(CH[chans[0]], len(chans), r0, BR),
                                       in_=v[1:97, :, 2:770])

                  def dma_single_o(ch, ap):
                      # direct [96, W] SBUF view -> one channel
                      ring().dma_start(out=out_ext[CH[ch], r0:r0 + BR, :], in_=ap)

                  def dma_single_e(ch, tile):
                      ring().dma_start(out=out_ext[CH[ch], r0:r0 + BR, :],
                                       in_=tile[1:97, 2:770].bitcast(f32))

                  # ---- conv helpers ----
                  def mm(grid, chain):
                      # chain: list of (band_idx, rhs_ap, dx)
                      if grid == 'E':
                          M, NH, bf, shift = ME, NE, bandE, PW - 2
                      elif grid == 'O':
                          M, NH, bf, shift = MO, NO, bandO, PW
                      else:
                          M, NH, bf, shift = MO, NO, bandB, 2
                      pst = pps.tile([ME, 1024], f32, tag="ps", name="ps")
                      ps = pst[:M]
                      for h in range(2):
                          for i, (bi, rhs, dx) in enumerate(chain):
                              nc.tensor.matmul(
                                  ps[:, h * 512:h * 512 + NH],
                                  bf(bi),
                                  rhs[:, shift + dx + h * NH: shift + dx + h * NH + NH],
                                  start=(i == 0), stop=(i == len(chain) - 1))
                      return ps

                  def convE(name, rhs=None):
                      r = (rhs if rhs is not None else strip)[:]
                      return mm('E', [(bi, r, dx) for dx, bi in P_E[name]])

                  def convO(name, rhs=None):
                      r = (rhs if rhs is not None else strip)[:]
                      return mm('O', [(bi, r, dx) for dx, bi in P_O[name]])

                  def convB(rhs_tile, band=0):
                      return mm('B', [(band, rhs_tile[:], dx) for dx, _ in P_B3])

                  def ps3(ps, NH):
                      return h3(ps[:, 0:1024])[:, :, 0:NH]

                  def e_roll(dtype=f32r):
                      return epool.tile([ME, EW], dtype, tag="ebuf", name="eb")

                  def o_new():
                      return opool.tile([MO, W], f32, tag="obuf", name="ob")

                  AX = mybir.AxisListType

                  def win(ap2d, base, n, k):
                      # overlapping sliding-window view [P, n, k] (stride-1)
                      c = ap2d[:, base:base + n].unsqueeze(2).copy()
                      p = [list(q) for q in c.ap]
                      c.ap = mybir.VecI64Pair([p[0], [1, n], [1, k]])
                      return c

                  # ================= E-grid stage (+ inline boxes) ===========
                  gx_ps = convE('gx')
                  gy_ps = convE('gy')
                  nc.scalar.copy(out=e3('gy'), in_=ps3(gy_ps, NE))
                  gyS = e2('gy')
                  gxx = wpool.tile([ME, EW], f32r, tag="gxx")
                  nc.scalar.activation(h3(gxx[:]), ps3(gx_ps, NE), AF.Square)
                  gyy = wpool.tile([ME, EW], f32r, tag="gyy")
                  nc.scalar.activation(gyy[:], gyS, AF.Square)
                  gxy = wpool.tile([ME, EW], f32r, tag="gxy")
                  nc.vector.tensor_mul(out=h3(gxy[:]), in0=ps3(gx_ps, NE),
                                       in1=e3('gy'))
                  nc.vector.tensor_scalar_mul(out=gxy[:, 1:2], in0=gxy[:, 1:2],
                                              scalar1=-1.0)
                  nc.vector.tensor_scalar_mul(out=gxy[:, 770:771],
                                              in0=gxy[:, 770:771], scalar1=-1.0)
                  absgx = e_roll()
                  nc.scalar.activation(h3(absgx[:]), ps3(gx_ps, NE), AF.Abs)
                  nc.vector.tensor_copy(out=e3('gx'), in_=ps3(gx_ps, NE))
                  absgy = e_roll()
                  nc.scalar.activation(absgy[:], gyS, AF.Abs)
                  dgd = e_roll()
                  nc.gpsimd.tensor_sub(out=dgd[:], in0=absgx[:].bitcast(f32),
                                       in1=absgy[:].bitcast(f32))
                  dma_single_e('dgd', dgd)
                  g2 = e_roll()
                  nc.gpsimd.tensor_add(out=g2[:], in0=gxx[:].bitcast(f32),
                                       in1=gyy[:].bitcast(f32))
                  gmag = e_roll()
                  nc.scalar.activation(gmag[:], g2[:].bitcast(f32), AF.Sqrt,
                                       bias=EPS)
                  dma_single_e('gmag', gmag)
                  dma_pack_e('egxy')

                  # structure tensor boxes (O psum pool; interleaves with E)
                  jxx_ps = convB(gxx)
                  jyy_ps = convB(gyy)
                  jyyS = o_new()
                  nc.scalar.copy(out=h3(jyyS[:]), in_=ps3(jyy_ps, NO))
                  tr = wpool.tile([MO, W], f32, tag="trT")
                  nc.vector.tensor_add(out=h3(tr[:]), in0=ps3(jxx_ps, NO),
                                       in1=h3(jyyS[:]))
                  dma_single_o('gen', tr[:])
                  dd = wpool.tile([MO, W], f32, tag="ddT")
                  nc.vector.tensor_sub(out=h3(dd[:]), in0=ps3(jxx_ps, NO),
                                       in1=h3(jyyS[:]))
                  jxy_ps = convB(gxy, band=1 + blk)
                  jxyS = o_new()
                  nc.scalar.copy(out=h3(jxyS[:]), in_=ps3(jxy_ps, NO))
                  d2 = o_new()
                  nc.scalar.activation(d2[:], dd[:], AF.Square)
                  jxy2 = o_new()
                  nc.scalar.activation(jxy2[:], jxyS[:], AF.Square)
                  ss = o_new()
                  nc.vector.scalar_tensor_tensor(out=ss[:], in0=jxy2[:],
                                                 scalar=4.0, in1=d2[:],
                                                 op0=A.mult, op1=A.add)
                  lam = o_new()
                  nc.scalar.activation(lam[:], ss[:], AF.Sqrt, bias=EPS)
                  tre = o_new()
                  nc.vector.tensor_scalar_add(tre[:], tr[:], EPS)
                  rtr = o_new()
                  nc.vector.reciprocal(rtr[:], tre[:])
                  coh = o_new()
                  nc.gpsimd.tensor_mul(out=coh[:], in0=lam[:], in1=rtr[:])
                  dma_single_o('coherence', coh[:])
                  aniso = o_new()
                  nc.gpsimd.tensor_mul(out=aniso[:], in0=dd[:], in1=rtr[:])
                  dma_single_o('anisotropy', aniso[:])

                  # checker / stripe / residual energies
                  chk_ps = convE('chk')
                  chksq = e_roll()
                  nc.scalar.activation(h3(chksq[:]), ps3(chk_ps, NE), AF.Square)
                  nc.scalar.copy(out=e3('chk'), in_=ps3(chk_ps, NE))
                  chk_e = o_new()
                  nc.scalar.copy(out=h3(chk_e[:]), in_=ps3(convB(chksq), NO))
                  dma_single_o('chk_e', chk_e[:])
                  stx_ps = convE('stx')
                  stxsq = e_roll()
                  nc.scalar.activation(h3(stxsq[:]), ps3(stx_ps, NE), AF.Square)
                  nc.scalar.copy(out=e3('stx'), in_=ps3(stx_ps, NE))
                  sty_ps = convE('sty')
                  stysq = e_roll()
                  nc.scalar.activation(h3(stysq[:]), ps3(sty_ps, NE), AF.Square)
                  nc.scalar.copy(out=e3('sty'), in_=ps3(sty_ps, NE))
                  s2 = e_roll()
                  nc.gpsimd.tensor_add(out=s2[:], in0=stxsq[:].bitcast(f32),
                                       in1=stysq[:].bitcast(f32))
                  dma_pack_e('estc')
                  if blk == 0:
                      # constant mask channels: 7 stride-0 broadcast DMAs
                      # from a [14,768] SBUF tile (write-only HBM traffic)
                      for mc in range(7):
                          msrc = m14_t[2 * mc:2 * mc + 2, 0:W].copy()
                          mp = [list(q) for q in msrc.ap]
                          msrc.ap = mybir.VecI64Pair([mp[0], [0, 96], [1, W]])
                          mdst = out_ext[mc:mc + 1, 0:1, 0:W].copy()
                          mdst.ap = mybir.VecI64Pair([[W, 2], [2 * W, 96],
                                                      [1, W]])
                          nc.sync.dma_start(out=mdst, in_=msrc)
                  # ---- line variances: exact f32 row ops (h) + hi/lo
                  # split fp32r conv (v) -- cancellation-sensitive via dconf.
                  sqT1 = wpool.tile([MO, SW], f32, tag="sqT1")
                  nc.scalar.activation(sqT1[:], T1[:], AF.Square)
                  msum = o_new()
                  nc.vector.tensor_reduce(out=msum[:], in_=win(T1[:], 4, W, 5),
                                          axis=AX.X, op=A.add)
                  mh2 = wpool.tile([MO, W], f32, tag="ddT")
                  nc.scalar.activation(mh2[:], msum[:], AF.Square, scale=0.2)
                  qsum = o_new()
                  nc.vector.tensor_reduce(out=qsum[:], in_=win(sqT1[:], 4, W, 5),
                                          axis=AX.X, op=A.add)
                  nc.vector.scalar_tensor_tensor(out=o2('lvh'), in0=qsum[:],
                                                 scalar=0.2, in1=mh2[:],
                                                 op0=A.mult, op1=A.subtract)
                  # vertical: hi/lo split banded conv
                  xh_t = wpool.tile([KIN, SW], f32r, tag="xh")
                  nc.scalar.copy(out=xh_t[:], in_=stripf)
                  xl_t = wpool.tile([KIN, SW], f32r, tag="xl")
                  nc.vector.tensor_sub(out=xl_t[:], in0=stripf,
                                       in1=xh_t[:].bitcast(f32))
                  bsq32 = wpool.tile([KIN, SW], f32, tag="bsq32")
                  nc.scalar.activation(bsq32[:], stripf, AF.Square)
                  bsql = wpool.tile([KIN, SW], f32r, tag="bsql")
                  nc.vector.tensor_sub(out=bsql[:], in0=bsq32[:],
                                       in1=bsqr[:].bitcast(f32))

                  def conv_split(xh, xl):
                      (dxh, bih), = P_O['avgv5_h']
                      (dxl, bil), = P_O['avgv5_l']
                      return mm('O', [(bih, xh[:], dxh), (bih, xl[:], dxh),
                                      (bil, xh[:], dxl)])

                  mv_ps = conv_split(xh_t, xl_t)
                  mv2 = o_new()
                  nc.scalar.activation(h3(mv2[:]), ps3(mv_ps, NO), AF.Square)
                  qv_ps = conv_split(bsqr, bsql)
                  nc.vector.scalar_tensor_tensor(out=o3('lvv'), in0=h3(mv2[:]),
                                                 scalar=-1.0, in1=ps3(qv_ps, NO),
                                                 op0=A.mult, op1=A.add)
                  nc.vector.tensor_sub(out=o2('lvd'), in0=o2('lvh'), in1=o2('lvv'))
                  alvd = o_new()
                  nc.scalar.activation(alvd[:], o2('lvd'), AF.Abs)
                  d2e = o_new()
                  nc.vector.scalar_tensor_tensor(out=d2e[:], in0=o2('lvh'),
                                                 scalar=EPS, in1=o2('lvv'),
                                                 op0=A.add, op1=A.add)
                  rden2 = o_new()
                  nc.vector.reciprocal(rden2[:], d2e[:])
                  nc.gpsimd.tensor_mul(out=o2('dconf'), in0=alvd[:], in1=rden2[:])
                  dma_pack_o('p3033')
                  # ---- smooth5 fills (polyphase, 1/den folded into bands) ----
                  strip_pp = strip[:].rearrange("k (c t) -> k c t", t=2)

                  def sm5_conv(groups):
                      ps = pps.tile([ME, 1024], f32, tag="ps", name="ps")[:MO]
                      for p in range(2):
                          chain = [(bi, j0, cX) for passes, cX in groups
                                   for (pp_, bi, j0) in passes if pp_ == p]
                          for i, (bi, j0, cX) in enumerate(chain):
                              nc.tensor.matmul(
                                  ps[:, p * 512:p * 512 + NO],
                                  bandO(bi), strip_pp[:, j0:j0 + NO, cX],
                                  start=(i == 0), stop=(i == len(chain) - 1))
                      return ps

                  def sub_interleaved(ch, ps, baseS):
                      t = o_new()
                      tv = t[:].rearrange("h (j t) -> h t j", t=2)
                      for ph in range(2):
                          nc.vector.tensor_sub(out=tv[:, ph, :],
                                               in0=ps[:, ph * 512:ph * 512 + NO],
                                               in1=baseS[:, ph * NO:(ph + 1) * NO])
                      dma_single_o(ch, t[:])

                  gf_ps = sm5_conv([(P_SM5_GF[0], 1), (P_SM5_GF[1], 0)])
                  gfS = o_new()
                  nc.vector.tensor_copy(out=h3(gfS[:]), in_=ps3(gf_ps, NO))
                  gbf_ps = sm5_conv([P_SM5['gbf']])
                  gbfS = o_new()
                  nc.vector.tensor_copy(out=h3(gbfS[:]), in_=ps3(gbf_ps, NO))
                  rf_ps = sm5_conv([P_SM5['rf']])
                  sub_interleaved('rg', rf_ps, gfS[:])
                  bf_ps = sm5_conv([P_SM5['bf']])
                  sub_interleaved('bg', bf_ps, gfS[:])
                  grf_ps = sm5_conv([P_SM5['grf']])
                  sub_interleaved('gpd', grf_ps, gbfS[:])
                  str_e = o_new()
                  nc.scalar.copy(out=h3(str_e[:]), in_=ps3(convB(s2), NO))
                  dma_single_o('str_e', str_e[:])

                  resh_ps = convE('resh')
                  rhsq = e_roll()
                  nc.scalar.activation(h3(rhsq[:]), ps3(resh_ps, NE), AF.Square)
                  reshS = e_roll()
                  nc.vector.tensor_copy(out=h3(reshS[:]), in_=ps3(resh_ps, NE))
                  res_eh = o_new()
                  nc.vector.tensor_copy(out=h3(res_eh[:]), in_=ps3(convB(rhsq), NO))
                  dma_single_o('res_eh', res_eh[:])
                  dma_single_e('res_h', reshS)
                  resv_ps = convE('resv')
                  rvsq = e_roll()
                  nc.scalar.activation(h3(rvsq[:]), ps3(resv_ps, NE), AF.Square)
                  resvS = e_roll()
                  nc.vector.tensor_copy(out=h3(resvS[:]), in_=ps3(resv_ps, NE))
                  res_ev = o_new()
                  nc.vector.tensor_copy(out=h3(res_ev[:]), in_=ps3(convB(rvsq), NO))
                  dma_single_o('res_ev', res_ev[:])
                  dma_single_e('res_v', resvS)

                  # ---- mgrad ----
                  v1 = wpool.tile([MO, SW], f32, tag="mgtmp")
                  nc.vector.tensor_max(out=v1[:], in0=T0e[:MO, :], in1=T2[:])
                  v3 = wpool.tile([MO, SW], f32, tag="v3")
                  nc.vector.tensor_max(out=v3[:], in0=v1[:], in1=T1[:])
                  n1 = wpool.tile([MO, SW], f32, tag="mgtmp")
                  nc.vector.tensor_tensor(out=n1[:], in0=T0e[:MO, :], in1=T2[:],
                                          op=A.min)
                  n3 = wpool.tile([MO, SW], f32, tag="n3")
                  nc.vector.tensor_tensor(out=n3[:], in0=n1[:], in1=T1[:],
                                          op=A.min)
                  wa = o_new()
                  nc.vector.tensor_max(out=wa[:], in0=v3[:, 4:4 + W],
                                       in1=v3[:, 6:6 + W])
                  wmx = o_new()
                  nc.vector.tensor_max(out=wmx[:], in0=wa[:], in1=v3[:, 5:5 + W])
                  na = o_new()
                  nc.vector.tensor_tensor(out=na[:], in0=n3[:, 4:4 + W],
                                          in1=n3[:, 6:6 + W], op=A.min)
                  nmn = o_new()
                  nc.vector.tensor_tensor(out=nmn[:], in0=na[:],
                                          in1=n3[:, 5:5 + W], op=A.min)
                  mgrad = o_new()
                  nc.vector.tensor_sub(out=mgrad[:], in0=wmx[:], in1=nmn[:])
                  dma_single_o('mgrad', mgrad[:])

                  # ---- O-grid row kernels on PE ----
                  lap_ps = convO('lap')
                  nc.scalar.copy(out=o3('lap'), in_=ps3(lap_ps, NO))
                  hxx_ps = convO('hxx')
                  abshxx = o_new()
                  nc.scalar.activation(h3(abshxx[:]), ps3(hxx_ps, NO), AF.Abs)
                  nc.scalar.copy(out=o3('hxx'), in_=ps3(hxx_ps, NO))
                  hyy_ps = convO('hyy')
                  abshyy = o_new()
                  nc.scalar.activation(h3(abshyy[:]), ps3(hyy_ps, NO), AF.Abs)
                  nc.scalar.copy(out=o3('hyy'), in_=ps3(hyy_ps, NO))
                  dsd = o_new()
                  nc.gpsimd.tensor_sub(out=dsd[:], in0=abshxx[:], in1=abshyy[:])
                  dma_single_o('dsd', dsd[:])
                  nc.scalar.copy(out=o3('hxy'), in_=ps3(convO('hxy'), NO))
                  dma_pack_o('p0711')
                  gdmS = o_new()
                  nc.scalar.copy(out=h3(gdmS[:]), in_=ps3(convO('gdm'), NO))
                  dma_single_o('gdm', gdmS[:])
                  gdaS = o_new()
                  nc.scalar.copy(out=h3(gdaS[:]), in_=ps3(convO('gda'), NO))
                  dma_single_o('gda', gdaS[:])

                  hshS = o_new()
                  nc.scalar.copy(out=h3(hshS[:]), in_=ps3(convO('hsh'), NO))
                  dma_single_o('hsh', hshS[:])
                  hsvS = o_new()
                  nc.scalar.copy(out=h3(hsvS[:]), in_=ps3(convO('hsv'), NO))
                  dma_single_o('hsv', hsvS[:])
                  hgh_ps = convO('hgh')
                  nc.vector.tensor_copy(out=o3('hgh'), in_=ps3(hgh_ps, NO))
                  hghS = o2('hgh')
                  hgv_ps = convO('hgv')
                  nc.vector.tensor_copy(out=o3('hgv'), in_=ps3(hgv_ps, NO))
                  hgvS = o2('hgv')
                  hd = o_new()
                  nc.vector.tensor_sub(out=hd[:], in0=hghS, in1=hgvS)
                  nc.scalar.activation(o2('ha_dis'), hd[:], AF.Abs)
                  dma_pack_o('p2123')

                  # ---- late conv outputs ----
                  nc.scalar.copy(out=o3('g45'), in_=ps3(convO('g45'), NO))
                  nc.scalar.copy(out=o3('g135'), in_=ps3(convO('g135'), NO))
                  nc.scalar.copy(out=o3('dctp'), in_=ps3(convO('dct'), NO))
                  dma_pack_o('p4446')

                  # ---- local mean / variance ----
                  lm_ps = convO('box3')
                  lmS = o_new()
                  nc.scalar.copy(out=h3(lmS[:]), in_=ps3(lm_ps, NO))
                  dma_single_o('lmean', lmS[:])
                  lm2 = o_new()
                  nc.scalar.activation(lm2[:], lmS[:], AF.Square)
                  lq_ps = convO('box3', rhs=bsqr)
                  lvar = o_new()
                  nc.vector.scalar_tensor_tensor(out=h3(lvar[:]), in0=h3(lm2[:]),
                                                 scalar=-1.0, in1=ps3(lq_ps, NO),
                                                 op0=A.mult, op1=A.add)
                  dma_single_o('lvar', lvar[:])


                  # ---- mhc ----
                  mhcf_ps = convO('mhc')
                  bmf = o_new()
                  nc.vector.tensor_sub(out=h3(bmf[:]), in0=h3(bayerO),
                                       in1=ps3(mhcf_ps, NO))
                  rres = o_new()
                  nc.gpsimd.tensor_mul(out=rres[:], in0=bmf[:],
                                       in1=rbg_t[:, 0:W])
                  dma_single_o('rres', rres[:])
                  bres = o_new()
                  nc.gpsimd.tensor_mul(out=bres[:], in0=bmf[:],
                                       in1=rbg_t[:, W:2 * W])
                  dma_single_o('bres', bres[:])
                  t1g = o_new()
                  nc.vector.tensor_sub(out=t1g[:], in0=bmf[:], in1=rres[:])
                  gbm = o_new()
                  nc.gpsimd.tensor_sub(out=gbm[:], in0=t1g[:], in1=bres[:])
                  mhc = o_new()
                  nc.vector.tensor_add(out=h3(mhc[:]), in0=ps3(mhcf_ps, NO),
                                       in1=h3(gbm[:]))
                  dma_single_o('mhc', mhc[:])
                  mha1 = o_new()
                  nc.vector.scalar_tensor_tensor(out=mha1[:], in0=hghS,
                                                 scalar=-0.5, in1=mhc[:],
                                                 op0=A.mult, op1=A.add)
                  mhc_ha = o_new()
                  nc.vector.scalar_tensor_tensor(out=mhc_ha[:], in0=hgvS,
                                                 scalar=-0.5, in1=mha1[:],
                                                 op0=A.mult, op1=A.add)
                  dma_single_o('mhc_ha', mhc_ha[:])


    nc.compile()
    return nc


def _get_program(loop=1, timing=False):
    key = (loop, timing)
    if key not in _PROGRAM:
        _PROGRAM[key] = _build_program(loop, timing)
    return _PROGRAM[key]


def _host_constants():
    def kmajor(b):
        n, k, m = b.shape
        return np.ascontiguousarray(np.transpose(b, (1, 0, 2)).reshape(k, n * m))

    consts = {
        "bandsO": kmajor(BANDS_O),
        "bandsE": kmajor(BANDS_E),
        "bandsB": kmajor(BANDS_B),
    }
    rbg = np.zeros((MO, 2 * W), np.float32)
    rbg[:, 0:W] = _tile_pattern(_mask_pattern('r'), MO, W)
    rbg[:, W:2 * W] = _tile_pattern(_mask_pattern('b'), MO, W)
    consts["rbgmask"] = rbg
    m14 = np.zeros((14, W), np.float32)
    for i, nm in enumerate(['r', 'g', 'b', 'gr', 'gb', 'row', 'col']):
        m14[2 * i:2 * i + 2] = _tile_pattern(_mask_pattern(nm), 2, W)
    consts["masks14"] = m14
    return consts


def _in_maps(bayer):
    consts = _host_constants()

    def kmajor(bnd):
        n, k, mm = bnd.shape
        return np.ascontiguousarray(np.transpose(bnd, (1, 0, 2)).reshape(k, n * mm))

    padded = np.pad(bayer[:, 0], ((0, 0), (PH, PH), (PW, PW)), mode='reflect')
    in_maps = []
    for c in range(NCORES):
        b, j = divmod(c, CORES_PER_BATCH)
        strip = padded[b, j * RPC: j * RPC + SR, :]
        m = dict(consts)
        m["xs"] = np.ascontiguousarray(strip)
        if j == 0 or j == CORES_PER_BATCH - 1:
            bb = np.stack([_BB_STD,
                           _bb_variant('top') if j == 0 else _BB_STD,
                           _bb_variant('bot') if j == CORES_PER_BATCH - 1 else _BB_STD])
            m["bandsB"] = kmajor(bb)
        in_maps.append(m)
    return in_maps


def kernel(bayer: np.ndarray) -> np.ndarray:
    from concourse.bass_utils import run_bass_kernel_spmd

    bayer = np.asarray(bayer, np.float32)
    assert bayer.shape == (B, 1, H, W), bayer.shape
    nc = _get_program()
    res = run_bass_kernel_spmd(nc, _in_maps(bayer), list(range(NCORES)))
    out = np.zeros((B, 52, H, W), np.float32)
    for c in range(NCORES):
        b, j = divmod(c, CORES_PER_BATCH)
        out[b, :, j * RPC:(j + 1) * RPC, :] = res.results[c]["out"]
    return out


# revision 51
# speedup vs baseline: 1.8111x; 1.8111x over previous
"""Trainium2 Bass kernel for nn_BayerFeatureExtractor.

Computes 52 feature channels from a [2,1,768,768] bayer image, data-parallel
over 8 NeuronCores (each core: one batch image x 192 rows, 2 row-blocks).

Strategy (v2, DMA/engine-balanced):
  - Host reflect-pads each batch image by (3 rows, 6 cols); each core gets a
    [198, 780] fp32r strip (rows on SBUF partitions).
  - ALL linear filters (2D stencils, 1xN/Nx1 kernels, 3x3 boxes, polyphase
    masked smooth5) run on the TensorEngine as banded matmuls: contraction
    over input rows with a banded weight matrix encoding the kernel's column
    profile; one PSUM-accumulated pass per nonzero kernel column with the
    moving operand shifted along the free dim.
  - 3x3 boxes of intermediates run on an "E" grid extended by 1 row/col so
    the second conv stage needs no partition-offset reads; reflect behavior
    at borders is exact (profiles symmetric or squared); gxy's antisymmetric
    reflect fixed by sign flips + per-edge-core band variants.
  - Pointwise math spread across Vector (DVE), Scalar (ACT), GPSIMD (Pool).
  - Morphological gradient via 3 row-shifted DMA copies + max/min chains.
  - Adjacent-channel groups that finish together are written into multi-
    channel SBUF pack tiles and go out in one strided-AP DMA each (row-major
    over channels); stragglers DMA individually straight from their compute
    buffers. The 7 constant mask channels go out in one DRAM->DRAM DMA from
    a host [7,192,768] tensor.
"""
import sys
import math

sys.path.insert(0, '/opt/trn_rl_repo')

import numpy as np

EPS = 1e-6

H = 768
W = 768
B = 2
NCORES = 8
CORES_PER_BATCH = 4
RPC = H // CORES_PER_BATCH          # 192 output rows per core
NBLK = 2
BR = RPC // NBLK                    # 96 output rows per block
PH = 3                              # host row padding
PW = 6                              # host col padding
SR = RPC + 2 * PH                   # 198 strip rows
SW = W + 2 * PW                     # 780 strip cols
KIN = BR + 2 * PH                   # 102 contraction rows per block
ME, MO = 98, 96                     # E-grid / O-grid matmul M
NE, NO = 386, 384                   # matmul half widths
EW = 772                            # E tile width (covers out cols -2..769)


# ---------------------------------------------------------------- kernels ---
def _npk(a, s=1.0):
    return np.asarray(a, dtype=np.float32) * np.float32(s)


def _gabor(theta, sigma=1.1, lambd=3.0, gamma=0.65):
    c = np.arange(-2, 3, dtype=np.float32)
    yy, xx = np.meshgrid(c, c, indexing='ij')
    xt = xx * math.cos(theta) + yy * math.sin(theta)
    yt = -xx * math.sin(theta) + yy * math.cos(theta)
    k = np.exp(-(xt ** 2 + gamma ** 2 * yt ** 2) / (2.0 * sigma ** 2)) * np.cos(
        2.0 * math.pi * xt / lambd)
    k = k - k.mean()
    return (k / max(np.abs(k).sum(), 1e-6)).astype(np.float32)


def _dct_like(u=2, v=2, size=5):
    c = np.arange(size, dtype=np.float32)
    yy, xx = np.meshgrid(c, c, indexing='ij')
    k = np.cos(math.pi * (2 * xx + 1) * u / (2 * size)) * np.cos(
        math.pi * (2 * yy + 1) * v / (2 * size))
    k = k - k.mean()
    return (k / max(np.abs(k).sum(), 1e-6)).astype(np.float32)


K_LAP = _npk([[0, 1, 0], [1, -4, 1], [0, 1, 0]])
K_HXX = _npk([[1, -2, 1]])                     # 1x3 row kernel
K_HYY = _npk([[1], [-2], [1]])                 # 3x1 col kernel
K_HXY = _npk([[1, 0, -1], [0, 0, 0], [-1, 0, 1]], 0.25)
K_GX = _npk([[-1, 0, 1], [-2, 0, 2], [-1, 0, 1]], 0.125)
K_GY = _npk([[-1, -2, -1], [0, 0, 0], [1, 2, 1]], 0.125)
K_GDM = _npk([[-2, -1, 0], [-1, 0, 1], [0, 1, 2]], 0.125)
K_GDA = _npk([[0, 1, 2], [-1, 0, 1], [-2, -1, 0]], 0.125)
K_CHK = _npk([[1, -1, 1], [-1, 1, -1], [1, -1, 1]], 1.0 / 9.0)

HGH_V = np.array([-0.25, 0.5, 0.5, 0.5, -0.25], np.float32)
K_HSH = _npk([[-0.5, 0.0, 1.0, 0.0, -0.5]])
K_HSV = _npk([[-0.5], [0.0], [1.0], [0.0], [-0.5]])
K_HGH = HGH_V.reshape(1, 5)
K_HGV = HGH_V.reshape(5, 1)
K_MHC = _npk([[0, 0, -1, 0, 0], [0, 0, 2, 0, 0], [-1, 2, 4, 2, -1],
              [0, 0, 2, 0, 0], [0, 0, -1, 0, 0]], 0.125)
K_STX = _npk([[0.25, -1.0, 1.5, -1.0, 0.25]])
K_STY = K_STX.reshape(5, 1).copy()
K_G45 = _gabor(math.pi / 4.0)
K_G135 = _gabor(3.0 * math.pi / 4.0)
K_DCT = _dct_like()
K_SMOOTH5 = (_npk([[1, 2, 3, 2, 1], [2, 4, 6, 4, 2], [3, 6, 9, 6, 3],
                   [2, 4, 6, 4, 2], [1, 2, 3, 2, 1]]) / np.float32(81.0))
K_RESH = (np.eye(1, 5, 2, dtype=np.float32) - K_HGH)      # delta - hgh (1x5)
K_RESV = (np.eye(5, 1, -2, dtype=np.float32) - K_HGV)     # delta - hgv (5x1)
K_BOX3 = np.full((3, 3), 1.0 / 9.0, np.float32)
K_AVGH5 = np.full((1, 5), 0.2, np.float32)
K_AVGV5 = np.full((5, 1), 0.2, np.float32)


def _mask_pattern(name):
    # value at (row parity, col parity), gbrg pattern
    m = np.zeros((2, 2), np.float32)
    if name == 'r':
        m[1, 0] = 1.0
    elif name == 'b':
        m[0, 1] = 1.0
    elif name == 'gr':
        m[1, 1] = 1.0
    elif name == 'gb':
        m[0, 0] = 1.0
    elif name == 'g':
        m[0, 0] = 1.0; m[1, 1] = 1.0
    elif name == 'row':
        m[1, :] = 1.0
    elif name == 'col':
        m[:, 1] = 1.0
    return m


def _den_pattern(name):
    # conv(mask, SMOOTH5) is exactly 2x2-periodic (reflect == parity ext.)
    pat = _mask_pattern(name)
    g = np.zeros((16, 16), np.float32)
    for r in range(16):
        for c in range(16):
            g[r, c] = pat[r % 2, c % 2]
    out = np.zeros((2, 2), np.float32)
    for r in (6, 7):
        for c in (6, 7):
            acc = np.float32(0.0)
            for dy in range(5):
                for dx in range(5):
                    acc += K_SMOOTH5[dy, dx] * g[r + dy - 2, c + dx - 2]
            out[r % 2, c % 2] = acc
    return np.maximum(out, EPS)


def _tile_pattern(pat, rows, cols):
    out = np.zeros((rows, cols), np.float32)
    for rp in range(2):
        for cp in range(2):
            out[rp::2, cp::2] = pat[rp % 2, cp % 2]
    return out


# ------------------------------------------------------------ band builder ---
class Bands:
    """Dedup banded lhsT matrices per grid ('O' out rows, 'E' extended, 'B' box)."""

    def __init__(self):
        self.items = {'O': [], 'E': [], 'B': []}
        self.index = {}

    def get(self, grid, prof):
        key = (grid, tuple(np.round(np.asarray(prof, np.float64), 10)))
        if key in self.index:
            return self.index[key]
        prof = np.asarray(prof, np.float32)
        kh = len(prof)
        off = kh // 2
        if grid == 'O':
            m = np.zeros((KIN, MO), np.float32)
            for mm in range(MO):
                base = mm + PH - off
                for t in range(kh):
                    m[base + t, mm] = prof[t]
        elif grid == 'E':
            m = np.zeros((KIN, ME), np.float32)
            for ii in range(ME):
                base = ii + PH - 1 - off
                for t in range(kh):
                    m[base + t, ii] = prof[t]
        else:  # 'B': 3-row box applied to E tiles
            m = np.zeros((ME, MO), np.float32)
            for mm in range(MO):
                for t in range(kh):
                    m[mm + t, mm] = prof[t]
        idx = len(self.items[grid])
        self.items[grid].append(m)
        self.index[key] = idx
        return idx

    def passes(self, grid, K):
        K = np.atleast_2d(np.asarray(K, np.float32))
        kw = K.shape[1]
        out = []
        for dxi in range(kw):
            col = K[:, dxi]
            if np.any(col != 0.0):
                out.append((dxi - kw // 2, self.get(grid, col)))
        return out


_BANDS = Bands()

P_E = {
    'gx': _BANDS.passes('E', K_GX),
    'gy': _BANDS.passes('E', K_GY),
    'chk': _BANDS.passes('E', K_CHK),
    'sty': _BANDS.passes('E', K_STY),
    'resv': _BANDS.passes('E', K_RESV),
    'stx': _BANDS.passes('E', K_STX),
    'resh': _BANDS.passes('E', K_RESH),
}
P_O = {
    'hyy': _BANDS.passes('O', K_HYY),
    'hxy': _BANDS.passes('O', K_HXY),
    'gdm': _BANDS.passes('O', K_GDM),
    'gda': _BANDS.passes('O', K_GDA),
    'hsv': _BANDS.passes('O', K_HSV),
    'hgv': _BANDS.passes('O', K_HGV),
    'g45': _BANDS.passes('O', K_G45),
    'g135': _BANDS.passes('O', K_G135),
    'dct': _BANDS.passes('O', K_DCT),
    'mhc': _BANDS.passes('O', K_MHC),
    'avgv5': _BANDS.passes('O', K_AVGV5),
    'box3': _BANDS.passes('O', K_BOX3),
    'lap': _BANDS.passes('O', K_LAP),
    'hxx': _BANDS.passes('O', K_HXX),
    'hsh': _BANDS.passes('O', K_HSH),
    'hgh': _BANDS.passes('O', K_HGH),
    'avgh5': _BANDS.passes('O', K_AVGH5),
}
def _trunc10(a):
    b = np.asarray(a, np.float32).copy()
    v = b.view(np.uint32)
    v &= np.uint32(0xFFFFE000)
    return b


K_AVGV5_H = _trunc10(K_AVGV5)
K_AVGV5_L = (K_AVGV5 - K_AVGV5_H).astype(np.float32)
P_O['avgv5_h'] = _BANDS.passes('O', K_AVGV5_H)
P_O['avgv5_l'] = _BANDS.passes('O', K_AVGV5_L)
_BANDS.get('B', K_BOX3[:, 0])
P_B3 = [(-1, 0), (0, 0), (1, 0)]   # 3-pass box over E tiles (band 0 = std)

BANDS_E = np.stack(_BANDS.items['E'])          # [nE, 102, 98]

# --- polyphase smooth5 bands: row-parity mask and 1/den folded into bands.
_SM5_META = {'r': (1, 0), 'b': (0, 1), 'gr': (1, 1), 'gb': (0, 0)}


def _sm5_passes(X, D):
    rX, cX = _SM5_META[X]
    rden = 1.0 / _den_pattern(D)
    out = []
    for p in range(2):
        for dxi in range(5):
            if (p + dxi) % 2 != cX:
                continue
            m = np.zeros((KIN, MO), np.float32)
            for mm in range(MO):
                for t in range(5):
                    k = mm + 1 + t
                    if (k + 1) % 2 == rX:
                        m[k, mm] = K_SMOOTH5[t, dxi] * rden[mm % 2, p]
            key = ('SM5', X, D, p, dxi)
            if key not in _BANDS.index:
                _BANDS.index[key] = len(_BANDS.items['O'])
                _BANDS.items['O'].append(m)
            # rhs strided-view offset (in cX-parity column units)
            j0 = (PW + p + (dxi - 2) - cX) // 2
            out.append((p, _BANDS.index[key], j0))
    return out


P_SM5 = {
    'rf': (_sm5_passes('r', 'r'), 0),
    'bf': (_sm5_passes('b', 'b'), 1),
    'grf': (_sm5_passes('gr', 'gr'), 1),
    'gbf': (_sm5_passes('gb', 'gb'), 0),
}
P_SM5_GF = (_sm5_passes('gr', 'g'), _sm5_passes('gb', 'g'))  # accumulate both

BANDS_O = np.stack(_BANDS.items['O'])          # [nO, 102, 96]
_BB_STD = _BANDS.items['B'][0]
BANDS_B = np.stack([_BB_STD, _BB_STD, _BB_STD])   # std, top-slot, bot-slot


def _bb_variant(kind):
    m = _BB_STD.copy()
    if kind == 'top':
        m[0, 0] = -m[0, 0]
    else:
        m[ME - 1, MO - 1] = -m[ME - 1, MO - 1]
    return m


CH = {n: i for i, n in enumerate([
    'r', 'g', 'b', 'gr', 'gb', 'rowm', 'colm',
    'lap', 'hxx', 'hyy', 'hxy', 'mgrad', 'gx', 'gy', 'gdm', 'gda', 'gmag',
    'coherence', 'anisotropy', 'hsh', 'hsv', 'hgh', 'hgv', 'ha_dis',
    'res_h', 'res_v', 'res_eh', 'res_ev', 'dgd', 'dsd', 'lvh', 'lvv', 'lvd',
    'dconf', 'rg', 'bg', 'gpd', 'mhc', 'mhc_ha', 'rres', 'bres',
    'stx', 'sty', 'chk', 'g45', 'g135', 'dctp', 'chk_e', 'str_e', 'lmean',
    'lvar', 'gen'])}

# O-grid packs: contiguous channel runs whose slots complete close together.
O_PACKS = {
    'p0711': ['lap', 'hxx', 'hyy', 'hxy'],               # ch 7..10
    'p2123': ['hgh', 'hgv', 'ha_dis'],                   # ch 21..23
    'p3033': ['lvh', 'lvv', 'lvd', 'dconf'],             # ch 30..33
    'p4446': ['g45', 'g135', 'dctp'],                    # ch 44..46
}
E_PACKS = {
    'egxy': ['gx', 'gy'],            # ch 12,13
    'estc': ['stx', 'sty', 'chk'],   # ch 41,42,43
}


# ------------------------------------------------------------- bass program ---
_PROGRAM = {}


def _build_program(loop=1, timing=False):
    import concourse.bacc as bacc
    import concourse.mybir as mybir
    from concourse.tile import TileContext

    f32 = mybir.dt.float32
    f32r = mybir.dt.float32r
    A = mybir.AluOpType
    AF = mybir.ActivationFunctionType

    nc = bacc.Bacc("TRN2")

    if timing:
        def declare(name, shape, dtype, isOutput):
            return nc.dram_tensor(name, shape, dtype).ap()
        tin = nc.declare_dram_parameter("tin", [1, 4], mybir.dt.float32, isOutput=False)
        tout = nc.declare_dram_parameter("tout", [1, 4], mybir.dt.float32, isOutput=True)
    else:
        def declare(name, shape, dtype, isOutput):
            return nc.declare_dram_parameter(name, shape, dtype, isOutput=isOutput)

    def register_const(value):
        t = nc.alloc_sbuf_tensor(f"constf32-{value}", [128, 1], f32)
        nc.gpsimd.memset(t.ap(), value)
        nc.const_aps.aps[(f32, value)] = t.ap()

    register_const(EPS)
    nc.all_engine_barrier()

    nO, nE, nB = BANDS_O.shape[0], BANDS_E.shape[0], BANDS_B.shape[0]
    xs_ext = declare("xs", [SR, SW], f32r, isOutput=False)
    bo_ext = declare("bandsO", [KIN, nO * MO], f32r, isOutput=False)
    be_ext = declare("bandsE", [KIN, nE * ME], f32r, isOutput=False)
    bb_ext = declare("bandsB", [ME, nB * MO], f32r, isOutput=False)
    rbgmask_ext = declare("rbgmask", [MO, 2 * W], f32, isOutput=False)
    masks7_ext = declare("masks7", [7, RPC, W], f32, isOutput=False)
    out_ext = declare("out", [52, RPC, W], f32, isOutput=True)

    with TileContext(nc) as tc:
        with (
            tc.tile_pool(name="const", bufs=1) as cpool,
            tc.tile_pool(name="work", bufs=1) as wpool,
            tc.tile_pool(name="ebuf", bufs=5) as epool,
            tc.tile_pool(name="obuf", bufs=6) as opool,
            tc.tile_pool(name="pack", bufs=1) as kpool,
            tc.tile_pool(name="psum", bufs=4, space="PSUM") as pps,
        ):
            # ---- constants -> SBUF (once, scalar ring; E bands first since
            # the E convs are the first consumers) ----
            be_t = cpool.tile([KIN, nE * ME], f32r)
            nc.scalar.dma_start(out=be_t[:], in_=be_ext[:])
            bb_t = cpool.tile([ME, nB * MO], f32r)
            nc.scalar.dma_start(out=bb_t[:], in_=bb_ext[:])
            bo_t = cpool.tile([KIN, nO * MO], f32r)
            nc.scalar.dma_start(out=bo_t[:], in_=bo_ext[:])
            rbg_t = cpool.tile([MO, 2 * W], f32)
            nc.scalar.dma_start(out=rbg_t[:], in_=rbgmask_ext[:])

            def bandO(i):
                return bo_t[:, i * MO:(i + 1) * MO]

            def bandE(i):
                return be_t[:, i * ME:(i + 1) * ME]

            def bandB(i):
                return bb_t[:, i * MO:(i + 1) * MO]

            def h3(ap):
                return ap.rearrange("p (b n) -> p b n", b=2)

            if timing:
                nc.sync.dma_start(out=tout[:], in_=tin[:])

            # DRAM view iterating (row, channel, col) over `k` channels
            # starting at ch0 with unit channel stride.
            def out_rcw(ch0, k, r0, nr):
                c = out_ext[ch0:ch0 + 1, r0:r0 + nr, 0:W].copy()
                c.ap = mybir.VecI64Pair([[W, nr], [RPC * W, k], [1, W]])
                return c

            def ring():
                # all output DMAs on the SP ring: SP has no compute, so a
                # DMA blocked waiting for its producer doesn't stall an
                # engine's instruction stream (scalar/vector rings would).
                return nc.sync

            import contextlib
            loop_cm = tc.For_i(0, loop, 1) if loop > 1 else contextlib.nullcontext()
            with loop_cm:
              # ---- inputs for BOTH blocks up front (double-buffered tags)
              # so block-1 loads aren't queued behind block-0 output DMAs
              # on the SP ring.
              intiles = []
              for blk in range(NBLK):
                  r0 = blk * BR
                  strip = wpool.tile([KIN, SW], f32r, tag="strip", bufs=2)
                  nc.sync.dma_start(out=strip[:], in_=xs_ext[r0:r0 + KIN, :])
                  T0e = wpool.tile([ME, SW], f32, tag="T0e", bufs=2)
                  nc.sync.dma_start(out=T0e[:], in_=strip[2:2 + ME, :].bitcast(f32))
                  T1 = wpool.tile([MO, SW], f32, tag="T1", bufs=2)
                  nc.sync.dma_start(out=T1[:], in_=strip[3:3 + MO, :].bitcast(f32))
                  T2 = wpool.tile([MO, SW], f32, tag="T2", bufs=2)
                  nc.sync.dma_start(out=T2[:], in_=strip[4:4 + MO, :].bitcast(f32))
                  intiles.append((strip, T0e, T1, T2))

              for blk in range(NBLK):
                  r0 = blk * BR
                  strip, T0e, T1, T2 = intiles[blk]
                  stripf = strip[:].bitcast(f32)
                  bayerO = T1[:, PW:PW + W]

                  bsqr = wpool.tile([KIN, SW], f32r, tag="bsqr", bufs=2)
                  nc.scalar.activation(bsqr[:], stripf, AF.Square)

                  # ---- output packs & slot views ----
                  oslot = {}
                  opack_t = {}
                  for name, chans in O_PACKS.items():
                      t = kpool.tile([MO, len(chans) * W], f32, tag=name,
                                     name=name)
                      v = t[:].rearrange("p (c w) -> p c w", c=len(chans))
                      v3 = t[:].rearrange("p (c b n) -> p c b n",
                                          c=len(chans), b=2)
                      opack_t[name] = (t, chans)
                      for s, ch in enumerate(chans):
                          oslot[ch] = (v[:, s, :], v3[:, s, :, :])
                  eslot = {}
                  epack_t = {}
                  for name, chans in E_PACKS.items():
                      t = kpool.tile([ME, len(chans) * EW], f32, tag=name,
                                     name=name)
                      v = t[:].rearrange("p (c w) -> p c w", c=len(chans))
                      v3 = t[:].rearrange("p (c b n) -> p c b n",
                                          c=len(chans), b=2)
                      epack_t[name] = (t, chans)
                      for s, ch in enumerate(chans):
                          eslot[ch] = (v[:, s, :], v3[:, s, :, :])

                  def o2(ch):
                      return oslot[ch][0]

                  def o3(ch):
                      return oslot[ch][1]

                  def e2(ch):
                      return eslot[ch][0]

                  def e3(ch):
                      return eslot[ch][1]

                  def dma_pack_o(name):
                      t, chans = opack_t[name]
                      v = t[:].rearrange("p (c w) -> p c w", c=len(chans))
                      ring().dma_start(out=out_rcw(CH[chans[0]], len(chans), r0, BR),
                                       in_=v)

                  def dma_pack_e(name):
                      t, chans = epack_t[name]
                      v = t[:].rearrange("p (c w) -> p c w", c=len(chans))
                      ring().dma_start(out=out_rcw(CH[chans[0]], len(chans), r0, BR),
                                       in_=v[1:97, :, 2:770])

                  def dma_single_o(ch, ap):
                      # direct [96, W] SBUF view -> one channel
                      ring().dma_start(out=out_ext[CH[ch], r0:r0 + BR, :], in_=ap)

                  def dma_single_e(ch, tile):
                      ring().dma_start(out=out_ext[CH[ch], r0:r0 + BR, :],
                                       in_=tile[1:97, 2:770].bitcast(f32))

                  # ---- conv helpers ----
                  def mm(grid, chain):
                      # chain: list of (band_idx, rhs_ap, dx)
                      if grid == 'E':
                          M, NH, bf, shift = ME, NE, bandE, PW - 2
                      elif grid == 'O':
                          M, NH, bf, shift = MO, NO, bandO, PW
                      else:
                          M, NH, bf, shift = MO, NO, bandB, 2
                      pst = pps.tile([ME, 1024], f32, tag="ps", name="ps")
                      ps = pst[:M]
                      for h in range(2):
                          for i, (bi, rhs, dx) in enumerate(chain):
                              nc.tensor.matmul(
                                  ps[:, h * 512:h * 512 + NH],
                                  bf(bi),
                                  rhs[:, shift + dx + h * NH: shift + dx + h * NH + NH],
                                  start=(i == 0), stop=(i == len(chain) - 1))
                      return ps

                  def convE(name, rhs=None):
                      r = (rhs if rhs is not None else strip)[:]
                      return mm('E', [(bi, r, dx) for dx, bi in P_E[name]])

                  def convO(name, rhs=None):
                      r = (rhs if rhs is not None else strip)[:]
                      return mm('O', [(bi, r, dx) for dx, bi in P_O[name]])

                  def convB(rhs_tile, band=0):
                      return mm('B', [(band, rhs_tile[:], dx) for dx, _ in P_B3])

                  def ps3(ps, NH):
                      return h3(ps[:, 0:1024])[:, :, 0:NH]

                  def e_roll(dtype=f32r):
                      return epool.tile([ME, EW], dtype, tag="ebuf", name="eb")

                  def o_new():
                      return opool.tile([MO, W], f32, tag="obuf", name="ob")

                  AX = mybir.AxisListType

                  def win(ap2d, base, n, k):
                      # overlapping sliding-window view [P, n, k] (stride-1)
                      c = ap2d[:, base:base + n].unsqueeze(2).copy()
                      p = [list(q) for q in c.ap]
                      c.ap = mybir.VecI64Pair([p[0], [1, n], [1, k]])
                      return c

                  # ================= E-grid stage (+ inline boxes) ===========
                  gx_ps = convE('gx')
                  gy_ps = convE('gy')
                  nc.scalar.copy(out=e3('gy'), in_=ps3(gy_ps, NE))
                  gyS = e2('gy')
                  gxx = wpool.tile([ME, EW], f32r, tag="gxx")
                  nc.scalar.activation(h3(gxx[:]), ps3(gx_ps, NE), AF.Square)
                  gyy = wpool.tile([ME, EW], f32r, tag="gyy")
                  nc.scalar.activation(gyy[:], gyS, AF.Square)
                  gxy = wpool.tile([ME, EW], f32r, tag="gxy")
                  nc.vector.tensor_mul(out=h3(gxy[:]), in0=ps3(gx_ps, NE),
                                       in1=e3('gy'))
                  nc.vector.tensor_scalar_mul(out=gxy[:, 1:2], in0=gxy[:, 1:2],
                                              scalar1=-1.0)
                  nc.vector.tensor_scalar_mul(out=gxy[:, 770:771],
                                              in0=gxy[:, 770:771], scalar1=-1.0)
                  absgx = e_roll()
                  nc.scalar.activation(h3(absgx[:]), ps3(gx_ps, NE), AF.Abs)
                  nc.vector.tensor_copy(out=e3('gx'), in_=ps3(gx_ps, NE))
                  absgy = e_roll()
                  nc.scalar.activation(absgy[:], gyS, AF.Abs)
                  dgd = e_roll()
                  nc.gpsimd.tensor_sub(out=dgd[:], in0=absgx[:].bitcast(f32),
                                       in1=absgy[:].bitcast(f32))
                  dma_single_e('dgd', dgd)
                  g2 = e_roll()
                  nc.gpsimd.tensor_add(out=g2[:], in0=gxx[:].bitcast(f32),
                                       in1=gyy[:].bitcast(f32))
                  gmag = e_roll()
                  nc.scalar.activation(gmag[:], g2[:].bitcast(f32), AF.Sqrt,
                                       bias=EPS)
                  dma_single_e('gmag', gmag)
                  dma_pack_e('egxy')

                  # structure tensor boxes (O psum pool; interleaves with E)
                  jxx_ps = convB(gxx)
                  jyy_ps = convB(gyy)
                  jyyS = o_new()
                  nc.scalar.copy(out=h3(jyyS[:]), in_=ps3(jyy_ps, NO))
                  tr = wpool.tile([MO, W], f32, tag="trT")
                  nc.vector.tensor_add(out=h3(tr[:]), in0=ps3(jxx_ps, NO),
                                       in1=h3(jyyS[:]))
                  dma_single_o('gen', tr[:])
                  dd = wpool.tile([MO, W], f32, tag="ddT")
                  nc.vector.tensor_sub(out=h3(dd[:]), in0=ps3(jxx_ps, NO),
                                       in1=h3(jyyS[:]))
                  jxy_ps = convB(gxy, band=1 + blk)
                  jxyS = o_new()
                  nc.scalar.copy(out=h3(jxyS[:]), in_=ps3(jxy_ps, NO))
                  d2 = o_new()
                  nc.scalar.activation(d2[:], dd[:], AF.Square)
                  jxy2 = o_new()
                  nc.scalar.activation(jxy2[:], jxyS[:], AF.Square)
                  ss = o_new()
                  nc.vector.scalar_tensor_tensor(out=ss[:], in0=jxy2[:],
                                                 scalar=4.0, in1=d2[:],
                                                 op0=A.mult, op1=A.add)
                  lam = o_new()
                  nc.scalar.activation(lam[:], ss[:], AF.Sqrt, bias=EPS)
                  tre = o_new()
                  nc.vector.tensor_scalar_add(tre[:], tr[:], EPS)
                  rtr = o_new()
                  nc.vector.reciprocal(rtr[:], tre[:])
                  coh = o_new()
                  nc.gpsimd.tensor_mul(out=coh[:], in0=lam[:], in1=rtr[:])
                  dma_single_o('coherence', coh[:])
                  aniso = o_new()
                  nc.gpsimd.tensor_mul(out=aniso[:], in0=dd[:], in1=rtr[:])
                  dma_single_o('anisotropy', aniso[:])

                  # checker / stripe / residual energies
                  chk_ps = convE('chk')
                  chksq = e_roll()
                  nc.scalar.activation(h3(chksq[:]), ps3(chk_ps, NE), AF.Square)
                  nc.scalar.copy(out=e3('chk'), in_=ps3(chk_ps, NE))
                  chk_e = o_new()
                  nc.scalar.copy(out=h3(chk_e[:]), in_=ps3(convB(chksq), NO))
                  dma_single_o('chk_e', chk_e[:])
                  stx_ps = convE('stx')
                  stxsq = e_roll()
                  nc.scalar.activation(h3(stxsq[:]), ps3(stx_ps, NE), AF.Square)
                  nc.scalar.copy(out=e3('stx'), in_=ps3(stx_ps, NE))
                  sty_ps = convE('sty')
                  stysq = e_roll()
                  nc.scalar.activation(h3(stysq[:]), ps3(sty_ps, NE), AF.Square)
                  nc.scalar.copy(out=e3('sty'), in_=ps3(sty_ps, NE))
                  s2 = e_roll()
                  nc.gpsimd.tensor_add(out=s2[:], in0=stxsq[:].bitcast(f32),
                                       in1=stysq[:].bitcast(f32))
                  dma_pack_e('estc')
                  if blk == 0:
                      # constant mask channels: 7 short DRAM->DRAM DMAs so
                      # their exclusive DMA-engine holds interleave with
                      # early output traffic
                      for mc in range(7):
                          nc.sync.dma_start(out=out_ext[mc, :, :],
                                            in_=masks7_ext[mc])
                  # ---- line variances: exact f32 row ops (h) + hi/lo
                  # split fp32r conv (v) -- cancellation-sensitive via dconf.
                  sqT1 = wpool.tile([MO, SW], f32, tag="sqT1")
                  nc.scalar.activation(sqT1[:], T1[:], AF.Square)
                  msum = o_new()
                  nc.vector.tensor_reduce(out=msum[:], in_=win(T1[:], 4, W, 5),
                                          axis=AX.X, op=A.add)
                  mh2 = wpool.tile([MO, W], f32, tag="ddT")
                  nc.scalar.activation(mh2[:], msum[:], AF.Square, scale=0.2)
                  qsum = o_new()
                  nc.vector.tensor_reduce(out=qsum[:], in_=win(sqT1[:], 4, W, 5),
                                          axis=AX.X, op=A.add)
                  nc.vector.scalar_tensor_tensor(out=o2('lvh'), in0=qsum[:],
                                                 scalar=0.2, in1=mh2[:],
                                                 op0=A.mult, op1=A.subtract)
                  # vertical: hi/lo split banded conv
                  xh_t = wpool.tile([KIN, SW], f32r, tag="xh")
                  nc.scalar.copy(out=xh_t[:], in_=stripf)
                  xl_t = wpool.tile([KIN, SW], f32r, tag="xl")
                  nc.vector.tensor_sub(out=xl_t[:], in0=stripf,
                                       in1=xh_t[:].bitcast(f32))
                  bsq32 = wpool.tile([KIN, SW], f32, tag="bsq32")
                  nc.scalar.activation(bsq32[:], stripf, AF.Square)
                  bsql = wpool.tile([KIN, SW], f32r, tag="bsql")
                  nc.vector.tensor_sub(out=bsql[:], in0=bsq32[:],
                                       in1=bsqr[:].bitcast(f32))

                  def conv_split(xh, xl):
                      (dxh, bih), = P_O['avgv5_h']
                      (dxl, bil), = P_O['avgv5_l']
                      return mm('O', [(bih, xh[:], dxh), (bih, xl[:], dxh),
                                      (bil, xh[:], dxl)])

                  mv_ps = conv_split(xh_t, xl_t)
                  mv2 = o_new()
                  nc.scalar.activation(h3(mv2[:]), ps3(mv_ps, NO), AF.Square)
                  qv_ps = conv_split(bsqr, bsql)
                  nc.vector.scalar_tensor_tensor(out=o3('lvv'), in0=h3(mv2[:]),
                                                 scalar=-1.0, in1=ps3(qv_ps, NO),
                                                 op0=A.mult, op1=A.add)
                  nc.vector.tensor_sub(out=o2('lvd'), in0=o2('lvh'), in1=o2('lvv'))
                  alvd = o_new()
                  nc.scalar.activation(alvd[:], o2('lvd'), AF.Abs)
                  d2e = o_new()
                  nc.vector.scalar_tensor_tensor(out=d2e[:], in0=o2('lvh'),
                                                 scalar=EPS, in1=o2('lvv'),
                                                 op0=A.add, op1=A.add)
                  rden2 = o_new()
                  nc.vector.reciprocal(rden2[:], d2e[:])
                  nc.gpsimd.tensor_mul(out=o2('dconf'), in0=alvd[:], in1=rden2[:])
                  dma_pack_o('p3033')
                  # ---- smooth5 fills (polyphase, 1/den folded into bands) ----
                  strip_pp = strip[:].rearrange("k (c t) -> k c t", t=2)

                  def sm5_conv(groups):
                      ps = pps.tile([ME, 1024], f32, tag="ps", name="ps")[:MO]
                      for p in range(2):
                          chain = [(bi, j0, cX) for passes, cX in groups
                                   for (pp_, bi, j0) in passes if pp_ == p]
                          for i, (bi, j0, cX) in enumerate(chain):
                              nc.tensor.matmul(
                                  ps[:, p * 512:p * 512 + NO],
                                  bandO(bi), strip_pp[:, j0:j0 + NO, cX],
                                  start=(i == 0), stop=(i == len(chain) - 1))
                      return ps

                  def sub_interleaved(ch, ps, baseS):
                      t = o_new()
                      tv = t[:].rearrange("h (j t) -> h t j", t=2)
                      for ph in range(2):
                          nc.vector.tensor_sub(out=tv[:, ph, :],
                                               in0=ps[:, ph * 512:ph * 512 + NO],
                                               in1=baseS[:, ph * NO:(ph + 1) * NO])
                      dma_single_o(ch, t[:])

                  gf_ps = sm5_conv([(P_SM5_GF[0], 1), (P_SM5_GF[1], 0)])
                  gfS = o_new()
                  nc.vector.tensor_copy(out=h3(gfS[:]), in_=ps3(gf_ps, NO))
                  gbf_ps = sm5_conv([P_SM5['gbf']])
                  gbfS = o_new()
                  nc.vector.tensor_copy(out=h3(gbfS[:]), in_=ps3(gbf_ps, NO))
                  rf_ps = sm5_conv([P_SM5['rf']])
                  sub_interleaved('rg', rf_ps, gfS[:])
                  bf_ps = sm5_conv([P_SM5['bf']])
                  sub_interleaved('bg', bf_ps, gfS[:])
                  grf_ps = sm5_conv([P_SM5['grf']])
                  sub_interleaved('gpd', grf_ps, gbfS[:])
                  str_e = o_new()
                  nc.scalar.copy(out=h3(str_e[:]), in_=ps3(convB(s2), NO))
                  dma_single_o('str_e', str_e[:])

                  resh_ps = convE('resh')
                  rhsq = e_roll()
                  nc.scalar.activation(h3(rhsq[:]), ps3(resh_ps, NE), AF.Square)
                  reshS = e_roll()
                  nc.vector.tensor_copy(out=h3(reshS[:]), in_=ps3(resh_ps, NE))
                  res_eh = o_new()
                  nc.vector.tensor_copy(out=h3(res_eh[:]), in_=ps3(convB(rhsq), NO))
                  dma_single_o('res_eh', res_eh[:])
                  dma_single_e('res_h', reshS)
                  resv_ps = convE('resv')
                  rvsq = e_roll()
                  nc.scalar.activation(h3(rvsq[:]), ps3(resv_ps, NE), AF.Square)
                  resvS = e_roll()
                  nc.vector.tensor_copy(out=h3(resvS[:]), in_=ps3(resv_ps, NE))
                  res_ev = o_new()
                  nc.vector.tensor_copy(out=h3(res_ev[:]), in_=ps3(convB(rvsq), NO))
                  dma_single_o('res_ev', res_ev[:])
                  dma_single_e('res_v', resvS)

                  # ---- mgrad ----
                  v1 = wpool.tile([MO, SW], f32, tag="mgtmp")
                  nc.vector.tensor_max(out=v1[:], in0=T0e[:MO, :], in1=T2[:])
                  v3 = wpool.tile([MO, SW], f32, tag="v3")
                  nc.vector.tensor_max(out=v3[:], in0=v1[:], in1=T1[:])
                  n1 = wpool.tile([MO, SW], f32, tag="mgtmp")
                  nc.vector.tensor_tensor(out=n1[:], in0=T0e[:MO, :], in1=T2[:],
                                          op=A.min)
                  n3 = wpool.tile([MO, SW], f32, tag="n3")
                  nc.vector.tensor_tensor(out=n3[:], in0=n1[:], in1=T1[:],
                                          op=A.min)
                  wa = o_new()
                  nc.vector.tensor_max(out=wa[:], in0=v3[:, 4:4 + W],
                                       in1=v3[:, 6:6 + W])
                  wmx = o_new()
                  nc.vector.tensor_max(out=wmx[:], in0=wa[:], in1=v3[:, 5:5 + W])
                  na = o_new()
                  nc.vector.tensor_tensor(out=na[:], in0=n3[:, 4:4 + W],
                                          in1=n3[:, 6:6 + W], op=A.min)
                  nmn = o_new()
                  nc.vector.tensor_tensor(out=nmn[:], in0=na[:],
                                          in1=n3[:, 5:5 + W], op=A.min)
                  mgrad = o_new()
                  nc.vector.tensor_sub(out=mgrad[:], in0=wmx[:], in1=nmn[:])
                  dma_single_o('mgrad', mgrad[:])

                  # ---- O-grid row kernels on PE ----
                  lap_ps = convO('lap')
                  nc.scalar.copy(out=o3('lap'), in_=ps3(lap_ps, NO))
                  hxx_ps = convO('hxx')
                  abshxx = o_new()
                  nc.scalar.activation(h3(abshxx[:]), ps3(hxx_ps, NO), AF.Abs)
                  nc.scalar.copy(out=o3('hxx'), in_=ps3(hxx_ps, NO))
                  hyy_ps = convO('hyy')
                  abshyy = o_new()
                  nc.scalar.activation(h3(abshyy[:]), ps3(hyy_ps, NO), AF.Abs)
                  nc.scalar.copy(out=o3('hyy'), in_=ps3(hyy_ps, NO))
                  dsd = o_new()
                  nc.gpsimd.tensor_sub(out=dsd[:], in0=abshxx[:], in1=abshyy[:])
                  dma_single_o('dsd', dsd[:])
                  nc.scalar.copy(out=o3('hxy'), in_=ps3(convO('hxy'), NO))
                  dma_pack_o('p0711')
                  gdmS = o_new()
                  nc.scalar.copy(out=h3(gdmS[:]), in_=ps3(convO('gdm'), NO))
                  dma_single_o('gdm', gdmS[:])
                  gdaS = o_new()
                  nc.scalar.copy(out=h3(gdaS[:]), in_=ps3(convO('gda'), NO))
                  dma_single_o('gda', gdaS[:])

                  hshS = o_new()
                  nc.scalar.copy(out=h3(hshS[:]), in_=ps3(convO('hsh'), NO))
                  dma_single_o('hsh', hshS[:])
                  hsvS = o_new()
                  nc.scalar.copy(out=h3(hsvS[:]), in_=ps3(convO('hsv'), NO))
                  dma_single_o('hsv', hsvS[:])
                  hgh_ps = convO('hgh')
                  nc.vector.tensor_copy(out=o3('hgh'), in_=ps3(hgh_ps, NO))
                  hghS = o2('hgh')
                  hgv_ps = convO('hgv')
                  nc.vector.tensor_copy(out=o3('hgv'), in_=ps3(hgv_ps, NO))
                  hgvS = o2('hgv')
                  hd = o_new()
                  nc.vector.tensor_sub(out=hd[:], in0=hghS, in1=hgvS)
                  nc.scalar.activation(o2('ha_dis'), hd[:], AF.Abs)
                  dma_pack_o('p2123')

                  # ---- late conv outputs ----
                  nc.scalar.copy(out=o3('g45'), in_=ps3(convO('g45'), NO))
                  nc.scalar.copy(out=o3('g135'), in_=ps3(convO('g135'), NO))
                  nc.scalar.copy(out=o3('dctp'), in_=ps3(convO('dct'), NO))
                  dma_pack_o('p4446')

                  # ---- local mean / variance ----
                  lm_ps = convO('box3')
                  lmS = o_new()
                  nc.scalar.copy(out=h3(lmS[:]), in_=ps3(lm_ps, NO))
                  dma_single_o('lmean', lmS[:])
                  lm2 = o_new()
                  nc.scalar.activation(lm2[:], lmS[:], AF.Square)
                  lq_ps = convO('box3', rhs=bsqr)
                  lvar = o_new()
                  nc.vector.scalar_tensor_tensor(out=h3(lvar[:]), in0=h3(lm2[:]),
                                                 scalar=-1.0, in1=ps3(lq_ps, NO),
                                                 op0=A.mult, op1=A.add)
                  dma_single_o('lvar', lvar[:])


                  # ---- mhc ----
                  mhcf_ps = convO('mhc')
                  bmf = o_new()
                  nc.vector.tensor_sub(out=h3(bmf[:]), in0=h3(bayerO),
                                       in1=ps3(mhcf_ps, NO))
                  rres = o_new()
                  nc.gpsimd.tensor_mul(out=rres[:], in0=bmf[:],
                                       in1=rbg_t[:, 0:W])
                  dma_single_o('rres', rres[:])
                  bres = o_new()
                  nc.gpsimd.tensor_mul(out=bres[:], in0=bmf[:],
                                       in1=rbg_t[:, W:2 * W])
                  dma_single_o('bres', bres[:])
                  t1g = o_new()
                  nc.vector.tensor_sub(out=t1g[:], in0=bmf[:], in1=rres[:])
                  gbm = o_new()
                  nc.gpsimd.tensor_sub(out=gbm[:], in0=t1g[:], in1=bres[:])
                  mhc = o_new()
                  nc.vector.tensor_add(out=h3(mhc[:]), in0=ps3(mhcf_ps, NO),
                                       in1=h3(gbm[:]))
                  dma_single_o('mhc', mhc[:])
                  mha1 = o_new()
                  nc.vector.scalar_tensor_tensor(out=mha1[:], in0=hghS,
                                                 scalar=-0.5, in1=mhc[:],
                                                 op0=A.mult, op1=A.add)
                  mhc_ha = o_new()
                  nc.vector.scalar_tensor_tensor(out=mhc_ha[:], in0=hgvS,
                                                 scalar=-0.5, in1=mha1[:],
                                                 op0=A.mult, op1=A.add)
                  dma_single_o('mhc_ha', mhc_ha[:])


    nc.compile()
    return nc


def _get_program(loop=1, timing=False):
    key = (loop, timing)
    if key not in _PROGRAM:
        _PROGRAM[key] = _build_program(loop, timing)
    return _PROGRAM[key]


def _host_constants():
    def kmajor(b):
        n, k, m = b.shape
        return np.ascontiguousarray(np.transpose(b, (1, 0, 2)).reshape(k, n * m))

    consts = {
        "bandsO": kmajor(BANDS_O),
        "bandsE": kmajor(BANDS_E),
        "bandsB": kmajor(BANDS_B),
    }
    rbg = np.zeros((MO, 2 * W), np.float32)
    rbg[:, 0:W] = _tile_pattern(_mask_pattern('r'), MO, W)
    rbg[:, W:2 * W] = _tile_pattern(_mask_pattern('b'), MO, W)
    consts["rbgmask"] = rbg
    m7 = np.zeros((7, RPC, W), np.float32)
    for i, nm in enumerate(['r', 'g', 'b', 'gr', 'gb', 'row', 'col']):
        m7[i] = _tile_pattern(_mask_pattern(nm), RPC, W)
    consts["masks7"] = m7
    return consts


def _in_maps(bayer):
    consts = _host_constants()

    def kmajor(bnd):
        n, k, mm = bnd.shape
        return np.ascontiguousarray(np.transpose(bnd, (1, 0, 2)).reshape(k, n * mm))

    padded = np.pad(bayer[:, 0], ((0, 0), (PH, PH), (PW, PW)), mode='reflect')
    in_maps = []
    for c in range(NCORES):
        b, j = divmod(c, CORES_PER_BATCH)
        strip = padded[b, j * RPC: j * RPC + SR, :]
        m = dict(consts)
        m["xs"] = np.ascontiguousarray(strip)
        if j == 0 or j == CORES_PER_BATCH - 1:
            bb = np.stack([_BB_STD,
                           _bb_variant('top') if j == 0 else _BB_STD,
                           _bb_variant('bot') if j == CORES_PER_BATCH - 1 else _BB_STD])
            m["bandsB"] = kmajor(bb)
        in_maps.append(m)
    return in_maps


def kernel(bayer: np.ndarray) -> np.ndarray:
    from concourse.bass_utils import run_bass_kernel_spmd

    bayer = np.asarray(bayer, np.float32)
    assert bayer.shape == (B, 1, H, W), bayer.shape
    nc = _get_program()
    res = run_bass_kernel_spmd(nc, _in_maps(bayer), list(range(NCORES)))
    out = np.zeros((B, 52, H, W), np.float32)
    for c in range(NCORES):
        b, j = divmod(c, CORES_PER_BATCH)
        out[b, :, j * RPC:(j + 1) * RPC, :] = res.results[c]["out"]
    return out
